# revision 1
# baseline (speedup 1.0000x reference)
"""GMM negative log-likelihood on 8 TRN2 NeuronCores.

score[n, m] = wlog[m] - qf[n, m] factors exactly as F[n, :6] @ C[:6, m]
with features F = [1, x, y, x^2, xy, y^2], so the O(N*M) work is a
K=6 bf16 matmul on the TensorEngine.  Per 128-sample tile the [128,1024]
scores land in a 2-bank PSUM tile; columns [0,CA) get a true Exp with
fused row-sum on the scalar engine (in-place, accum_out), columns
[CA,1024) get a Schraudolph fast-exp on the vector engine (affine in
f32, int32 cast-on-write = exponent/mantissa bit construction, bitcast
back to f32, row-sum), splitting the exp work across both engines at
their throughput ratio.  Final ln+sum runs on the host in f64.

Inputs arrive as one [102, 3072] bf16 blob: the feature rows live at
partition groups {0,32,64,96} (PE quadrant-aligned) with C replicated
per group, so the DMA uses 24 partitions instead of 6 and the first
matmul can start after a ~2KB/partition leading transfer.
Data-parallel over N: each core gets 8192 samples and the full C.
"""

import numpy as np

import concourse.bacc as bacc
import concourse.bass as bass
import concourse.mybir as mybir
import concourse.tile as tile
from concourse.bass_utils import run_bass_kernel_spmd

N, M, NCORES = 65536, 1024, 8
NSH = N // NCORES          # 8192 samples per core
P = 128                    # partitions per tile
NT = NSH // P              # 64 tiles per core
NG = 4                     # partition groups for features
GSH = NSH // NG            # 2048 samples per group
NC_LOC = GSH // P          # 16 local col blocks per group
HALF = M // 2              # 512 = max moving free dim per matmul
BLOBW = M + GSH            # 3072 blob columns: [cmat | features]
CA = 660                   # columns handled by scalar-engine true exp
CD = M - CA                # Schraudolph columns (vector engine)

# Schraudolph: exp(s) ~= bitcast_f32(int32(A*s + B)), A = 2^23/ln2.
# B = 2^23*(127 - c) with c = log2(mean_f (1+f)*2^-f) making the
# relative error zero-mean over uniform mantissa fractions.
_SCH_A = float(2 ** 23 / np.log(2.0))
_SCH_C = float(np.log2(np.mean((1.0 + np.linspace(0, 1, 4097)) * 2.0 ** -np.linspace(0, 1, 4097))))
_SCH_B = float(2 ** 23 * (127.0 - _SCH_C))

_cache = {}


def _build(ca=CA):
    f32 = mybir.dt.float32
    i32 = mybir.dt.int32
    bf16 = mybir.dt.bfloat16
    nc = bacc.Bacc(None, target_bir_lowering=False)

    cd = M - ca
    blob_d = nc.declare_dram_parameter("blob", [102, BLOBW], bf16, isOutput=False)
    out_d = nc.declare_dram_parameter("out", [P, 2 * NT], f32, isOutput=True)

    with tile.TileContext(nc) as tc:
        with (
            tc.tile_pool(name="const", bufs=1) as const,
            tc.tile_pool(name="psa", bufs=2, space=bass.MemorySpace.PSUM) as psa,
            tc.tile_pool(name="psd", bufs=2, space=bass.MemorySpace.PSUM) as psd,
        ):
            blob = const.tile([102, BLOBW], bf16)
            # staged: [cmat | first col block] first so compute starts early
            nc.sync.dma_start(out=blob[:, 0:M + P], in_=blob_d[:, 0:M + P])
            nc.sync.dma_start(out=blob[:, M + P:M + 5 * P], in_=blob_d[:, M + P:M + 5 * P])
            nc.sync.dma_start(out=blob[:, M + 5 * P:BLOBW], in_=blob_d[:, M + 5 * P:BLOBW])

            sa = const.tile([P, NT], f32)  # ACT partial row-sums
            sd = const.tile([P, NT], f32)  # DVE partial row-sums

            # PE p-state warm-up: small data-independent matmuls keep the
            # tensor engine busy from t~0 so the real matmuls hit full clock
            # sooner; their target reuses a psum rotation slot that the real
            # loop only needs again ~2 tiles in.
            warm = const.tile([6, P], bf16)
            nc.vector.memset(warm[:], 0.0)
            wpt = psa.tile([P, ca], f32, tag="pa")
            for _ in range(16):
                nc.tensor.matmul(wpt[:, 0:P], warm[:], warm[:], tile_position=(0, 0))

            # ACT: true exp + fused row-sum on [0, ca), own per-tile psum
            # tile.  DVE: Schraudolph fast-exp on [ca, M) + row-sum, on a
            # pair-shared psum tile so both ops run double-width; the pair's
            # DVE work is issued in the next iteration (write-acks settled).
            pend = None  # (pair tile, first tile idx) awaiting DVE ops
            pd = None
            t = 0
            for c in range(NC_LOC):
                for g in range(NG):
                    gp = 32 * g
                    cmat = blob[gp:gp + 6, 0:M]
                    lhsT = blob[gp:gp + 6, M + c * P:M + (c + 1) * P]
                    tp = (gp, 0)
                    j = t % 2
                    if j == 0:
                        # bank-padded: each j slice starts on a PSUM bank
                        # boundary (matmul outputs must not cross banks)
                        pd = psd.tile([P, 2, HALF], f32, tag="pd")
                    pa = psa.tile([P, ca], f32, tag="pa")
                    nc.tensor.matmul(pa[:, 0:HALF], lhsT, cmat[:, 0:HALF],
                                     tile_position=tp)
                    nc.tensor.matmul(pa[:, HALF:ca], lhsT, cmat[:, HALF:ca],
                                     tile_position=tp)
                    nc.tensor.matmul(pd[:, j, 0:cd], lhsT, cmat[:, ca:M],
                                     tile_position=tp)
                    nc.scalar.activation(
                        pa[:], pa[:], mybir.ActivationFunctionType.Exp,
                        accum_out=sa[:, t:t + 1],
                    )
                    if j == 1:
                        if pend is not None:
                            ppd, pi = pend
                            sl_f = ppd[:, :, 0:cd]
                            nc.vector.tensor_scalar(
                                out=sl_f.bitcast(i32), in0=sl_f,
                                scalar1=_SCH_A, scalar2=_SCH_B,
                                op0=mybir.AluOpType.mult, op1=mybir.AluOpType.add,
                            )
                            nc.vector.reduce_sum(sd[:, pi:pi + 2], sl_f,
                                                 axis=mybir.AxisListType.X)
                        pend = (pd, t - 1)
                    t += 1
            ppd, pi = pend
            sl_f = ppd[:, :, 0:cd]
            nc.vector.tensor_scalar(
                out=sl_f.bitcast(i32), in0=sl_f, scalar1=_SCH_A, scalar2=_SCH_B,
                op0=mybir.AluOpType.mult, op1=mybir.AluOpType.add,
            )
            nc.vector.reduce_sum(sd[:, pi:pi + 2], sl_f, axis=mybir.AxisListType.X)

            # ship both partial-sum arrays; host adds + logs in f64.  The sa
            # DMA is issued first so its fixed DGE latency overlaps the final
            # DVE ops feeding sd.
            nc.sync.dma_start(out=out_d[:, 0:NT], in_=sa[:])
            nc.sync.dma_start(out=out_d[:, NT:2 * NT], in_=sd[:])

    nc.compile()
    return nc


def kernel(sample, mu, sigma_log, theta, w):
    import ml_dtypes

    x = sample[:, 0].astype(np.float64)
    y = sample[:, 1].astype(np.float64)
    mux = mu[:, 0].astype(np.float64)
    muy = mu[:, 1].astype(np.float64)
    sl = sigma_log.astype(np.float64)
    th = theta.astype(np.float64)
    wv = w[:, 0].astype(np.float64)

    a = np.exp(-2.0 * sl[:, 0])
    b = np.exp(-2.0 * sl[:, 1])
    c, s = np.cos(th), np.sin(th)
    g11 = a * c * c + b * s * s
    g12 = (a - b) * c * s
    g22 = a * s * s + b * c * c
    wmax = wv.max()
    wlog = (wv - (wmax + np.log(np.exp(wv - wmax).sum()))) - sl.sum(axis=1)

    # score = F @ C with F = [1, x, y, x^2, xy, y^2]
    cm = np.stack([
        wlog - (g11 * mux * mux + 2.0 * g12 * mux * muy + g22 * muy * muy),
        2.0 * (g11 * mux + g12 * muy),
        2.0 * (g12 * mux + g22 * muy),
        -g11,
        -2.0 * g12,
        -g22,
    ]).astype(np.float32)
    ftf = np.stack([np.ones_like(x), x, y, x * x, x * y, y * y]).astype(np.float32)

    cm16 = cm.astype(ml_dtypes.bfloat16)
    ftf16 = ftf.astype(ml_dtypes.bfloat16)

    if "nc" not in _cache:
        _cache["nc"] = _build()
    nc = _cache["nc"]

    in_maps = []
    for i in range(NCORES):
        blob = np.zeros((102, BLOBW), dtype=ml_dtypes.bfloat16)
        base = i * NSH
        for g in range(NG):
            gp = 32 * g
            blob[gp:gp + 6, 0:M] = cm16
            blob[gp:gp + 6, M:BLOBW] = ftf16[:, base + g * GSH:base + (g + 1) * GSH]
        in_maps.append({"blob": blob})
    res = run_bass_kernel_spmd(nc, in_maps, core_ids=list(range(NCORES)))
    _cache["last_result"] = res
    total = np.float64(0.0)
    for r in res.results:
        o = np.asarray(r["out"], dtype=np.float64)
        total += np.log(o[:, 0:NT] + o[:, NT:2 * NT]).sum()
    return np.float32(-total)



# revision 4
# speedup vs baseline: 1.1198x; 1.1198x over previous
"""GMM negative log-likelihood on 8 TRN2 NeuronCores.

score[n, m] = wlog[m] - qf[n, m] factors exactly as F[n, :6] @ C[:6, m]
with features F = [1, x, y, x^2, xy, y^2].  The kernel computes scores
TRANSPOSED: each matmul produces a [128 component, 1024 sample] PSUM
tile (lhsT = a 128-column block of C, moving operand = the feature
chunk), so the mixture sum over m runs on the TensorEngine: after the
exp pass writes E = exp(score) to SBUF as bf16, eight [128,128]^T @
ones[128,1] matmuls per tile reduce over the component partitions with
a single moving column each, landing per-sample partial sums back in
PSUM with samples on partitions.  This removes both the activation
accumulator read-out and the vector-engine TensorReduce of the old
row-major design; ACT and DVE spend all their cycles on the exp pass.

The exp pass splits each 1024-sample tile between the scalar engine
(true Exp, psum f32 -> sbuf bf16) and the vector engine (Schraudolph
fast-exp: affine in f32, int16 cast-on-write = bf16 exponent/mantissa
bit construction) in ratio ~537:487 matching their throughputs.

Inputs arrive as one [102, 3072] bf16 blob: feature rows at partition
groups {0,32,64,96} (PE quadrant-aligned) with C replicated per group.
Data-parallel over N: each core gets 8192 samples and the full C.
Host sums the 8 component-block partials per sample and takes log in
f64.
"""

import numpy as np

import concourse.bacc as bacc
import concourse.bass as bass
import concourse.mybir as mybir
import concourse.tile as tile
from concourse.bass_utils import run_bass_kernel_spmd

N, M, NCORES = 65536, 1024, 8
NSH = N // NCORES          # 8192 samples per core
P = 128                    # partitions per tile
NG = 4                     # partition groups for features
GSH = NSH // NG            # 2048 samples per group
NCH = 2                    # sample chunks per group
CH = GSH // NCH            # 1024 samples per chunk
NMB = M // P               # 8 component blocks
BLOBW = M + GSH            # 3072 blob columns: [cmat | features]
ACOLS = 537                # sample columns on the scalar engine (true exp)
HALF = 512                 # psum bank width in f32 / max moving free dim
NRED = NG * NCH * NMB * (CH // P)   # 512 reduce partial columns

# Schraudolph in bf16: exp(s) ~= bitcast_bf16(int16(A*s + B)), A = 2^7/ln2.
# B = 2^7*(127 - c) with c making the relative error zero-mean over
# uniform mantissa fractions.
_SCH_A = float(2 ** 7 / np.log(2.0))
_SCH_C = float(np.log2(np.mean((1.0 + np.linspace(0, 1, 4097)) * 2.0 ** -np.linspace(0, 1, 4097))))
_SCH_B = float(2 ** 7 * (127.0 - _SCH_C))

_cache = {}


def _build(acols=ACOLS):
    f32 = mybir.dt.float32
    i16 = mybir.dt.int16
    bf16 = mybir.dt.bfloat16
    nc = bacc.Bacc(None, target_bir_lowering=False)

    blob_d = nc.declare_dram_parameter("blob", [102, BLOBW], bf16, isOutput=False)
    out_d = nc.declare_dram_parameter("out", [P, NRED], f32, isOutput=True)

    with tile.TileContext(nc) as tc:
        with (
            tc.tile_pool(name="const", bufs=1) as const,
            tc.tile_pool(name="ps", bufs=3, space=bass.MemorySpace.PSUM) as ps,
            tc.tile_pool(name="red", bufs=1, space=bass.MemorySpace.PSUM) as redp,
            tc.tile_pool(name="esb", bufs=3) as esb,
        ):
            blob = const.tile([102, BLOBW], bf16)
            # staged: [cmat | chunk 0] first so compute starts early
            nc.sync.dma_start(out=blob[:, 0:M + CH], in_=blob_d[:, 0:M + CH])
            nc.sync.dma_start(out=blob[:, M + CH:BLOBW], in_=blob_d[:, M + CH:BLOBW])

            ones = const.tile([P, 1], bf16)
            nc.vector.memset(ones[:], 1.0)
            red = redp.tile([P, NRED], f32)  # one psum bank of partials

            # PE p-state warm-up: small data-independent matmuls keep the
            # tensor engine busy from t~0 so the real matmuls hit full clock
            # sooner; the target reuses a psum rotation slot that the main
            # loop only needs again ~3 tiles in.
            warm = const.tile([6, P], bf16)
            nc.vector.memset(warm[:], 0.0)
            wpt = ps.tile([P, CH], f32, tag="ps")
            for _ in range(12):
                nc.tensor.matmul(wpt[:, 0:P], warm[:], warm[:], tile_position=(0, 0))

            # Reduce matmuls for a finished tile are emitted one iteration
            # late so the PE never waits on the exp pass of the tile it is
            # reducing.
            pend = []

            def flush(entry):
                et, g, c, mb = entry
                base = ((g * NCH + c) * NMB + mb) * (CH // P)
                for j in range(CH // P):
                    nc.tensor.matmul(
                        red[:, base + j:base + j + 1],
                        et[:, j * P:(j + 1) * P],
                        ones[:],
                        tile_position=(0, 0),
                    )

            for c in range(NCH):
                for g in range(NG):
                    gp = 32 * g
                    for mb in range(NMB):
                        lhsT = blob[gp:gp + 6, mb * P:(mb + 1) * P]
                        fmov = blob[gp:gp + 6, M + c * CH:M + (c + 1) * CH]
                        pt = ps.tile([P, CH], f32, tag="ps")
                        nc.tensor.matmul(pt[:, 0:HALF], lhsT, fmov[:, 0:HALF],
                                         tile_position=(gp, 0))
                        nc.tensor.matmul(pt[:, HALF:CH], lhsT, fmov[:, HALF:CH],
                                         tile_position=(gp, 0))
                        et = esb.tile([P, CH], bf16, tag="e")
                        nc.scalar.activation(
                            et[:, 0:acols], pt[:, 0:acols],
                            mybir.ActivationFunctionType.Exp,
                        )
                        nc.vector.tensor_scalar(
                            out=et[:, acols:CH].bitcast(i16),
                            in0=pt[:, acols:CH],
                            scalar1=_SCH_A, scalar2=_SCH_B,
                            op0=mybir.AluOpType.mult, op1=mybir.AluOpType.add,
                        )
                        if pend:
                            flush(pend.pop())
                        pend.append((et, g, c, mb))
            flush(pend.pop())

            redsb = const.tile([P, NRED], f32)
            nc.vector.tensor_copy(redsb[:], red[:])
            nc.sync.dma_start(out=out_d[:], in_=redsb[:])

    nc.compile()
    return nc


def kernel(sample, mu, sigma_log, theta, w):
    import ml_dtypes

    x = sample[:, 0].astype(np.float64)
    y = sample[:, 1].astype(np.float64)
    mux = mu[:, 0].astype(np.float64)
    muy = mu[:, 1].astype(np.float64)
    sl = sigma_log.astype(np.float64)
    th = theta.astype(np.float64)
    wv = w[:, 0].astype(np.float64)

    a = np.exp(-2.0 * sl[:, 0])
    b = np.exp(-2.0 * sl[:, 1])
    c, s = np.cos(th), np.sin(th)
    g11 = a * c * c + b * s * s
    g12 = (a - b) * c * s
    g22 = a * s * s + b * c * c
    wmax = wv.max()
    wlog = (wv - (wmax + np.log(np.exp(wv - wmax).sum()))) - sl.sum(axis=1)

    # score = F @ C with F = [1, x, y, x^2, xy, y^2]
    cm = np.stack([
        wlog - (g11 * mux * mux + 2.0 * g12 * mux * muy + g22 * muy * muy),
        2.0 * (g11 * mux + g12 * muy),
        2.0 * (g12 * mux + g22 * muy),
        -g11,
        -2.0 * g12,
        -g22,
    ]).astype(np.float32)
    ftf = np.stack([np.ones_like(x), x, y, x * x, x * y, y * y]).astype(np.float32)

    cm16 = cm.astype(ml_dtypes.bfloat16)
    ftf16 = ftf.astype(ml_dtypes.bfloat16)

    if "nc" not in _cache:
        _cache["nc"] = _build()
    nc = _cache["nc"]

    in_maps = []
    for i in range(NCORES):
        blob = np.zeros((102, BLOBW), dtype=ml_dtypes.bfloat16)
        base = i * NSH
        for g in range(NG):
            gp = 32 * g
            blob[gp:gp + 6, 0:M] = cm16
            blob[gp:gp + 6, M:BLOBW] = ftf16[:, base + g * GSH:base + (g + 1) * GSH]
        in_maps.append({"blob": blob})
    res = run_bass_kernel_spmd(nc, in_maps, core_ids=list(range(NCORES)))
    _cache["last_result"] = res
    total = np.float64(0.0)
    for r in res.results:
        o = np.asarray(r["out"], dtype=np.float64)
        # columns ordered ((g, c, mb), j): sum the NMB partials per sample
        o = o.reshape(P, NG * NCH, NMB, CH // P)
        ssum = o.sum(axis=2)  # [P, g*c, j]
        total += np.log(ssum).sum()
    return np.float32(-total)


# revision 7
# speedup vs baseline: 1.1278x; 1.0071x over previous
"""GMM negative log-likelihood on 8 TRN2 NeuronCores.

score[n, m] = wlog[m] - qf[n, m] factors exactly as F[n, :6] @ C[:6, m]
with features F = [1, x, y, x^2, xy, y^2].  The kernel computes scores
TRANSPOSED: each matmul produces a [128 component, 1024 sample] PSUM
tile (lhsT = a 128-column block of C, moving operand = the feature
chunk), so the mixture sum over m runs on the TensorEngine: after the
exp pass writes E = exp(score) to SBUF as bf16, eight [128,128]^T @
ones[128,1] matmuls per tile reduce over the component partitions with
a single moving column each, landing per-sample partial sums back in
PSUM with samples on partitions.  This removes both the activation
accumulator read-out and the vector-engine TensorReduce of the old
row-major design; ACT and DVE spend all their cycles on the exp pass.

The exp pass splits each 1024-sample tile between the scalar engine
(true Exp, psum f32 -> sbuf bf16) and the vector engine (Schraudolph
fast-exp: affine in f32, int16 cast-on-write = bf16 exponent/mantissa
bit construction) in ratio ~537:487 matching their throughputs.

Inputs arrive as one [102, 3072] bf16 blob: feature rows at partition
groups {0,32,64,96} (PE quadrant-aligned) with C replicated per group.
Data-parallel over N: each core gets 8192 samples and the full C.
Host sums the 8 component-block partials per sample and takes log in
f64.
"""

import numpy as np

import concourse.bacc as bacc
import concourse.bass as bass
import concourse.mybir as mybir
import concourse.tile as tile
from concourse.bass_utils import run_bass_kernel_spmd

N, M, NCORES = 65536, 1024, 8
NSH = N // NCORES          # 8192 samples per core
P = 128                    # partitions per tile
NG = 4                     # partition groups for features
GSH = NSH // NG            # 2048 samples per group
NCH = 2                    # sample chunks per group
CH = GSH // NCH            # 1024 samples per chunk
NMB = M // P               # 8 component blocks
BLOBW = M + GSH            # 3072 blob columns: [cmat | features]
ACOLS = 537                # sample columns on the scalar engine (true exp)
HALF = 512                 # psum bank width in f32 / max moving free dim
NRED = NG * NCH * NMB * (CH // P)   # 512 reduce partial columns

# Schraudolph in bf16: exp(s) ~= bitcast_bf16(int16(A*s + B)), A = 2^7/ln2.
# B = 2^7*(127 - c) with c making the relative error zero-mean over
# uniform mantissa fractions.
_SCH_A = float(2 ** 7 / np.log(2.0))
_SCH_C = float(np.log2(np.mean((1.0 + np.linspace(0, 1, 4097)) * 2.0 ** -np.linspace(0, 1, 4097))))
_SCH_B = float(2 ** 7 * (127.0 - _SCH_C))

_cache = {}


def _build(acols=ACOLS):
    f32 = mybir.dt.float32
    i16 = mybir.dt.int16
    bf16 = mybir.dt.bfloat16
    nc = bacc.Bacc(None, target_bir_lowering=False)

    blob_d = nc.declare_dram_parameter("blob", [102, BLOBW], bf16, isOutput=False)
    out_d = nc.declare_dram_parameter("out", [P, NRED], f32, isOutput=True)

    with tile.TileContext(nc) as tc:
        with (
            tc.tile_pool(name="const", bufs=1) as const,
            tc.tile_pool(name="ps", bufs=3, space=bass.MemorySpace.PSUM) as ps,
            tc.tile_pool(name="red", bufs=1, space=bass.MemorySpace.PSUM) as redp,
            tc.tile_pool(name="esb", bufs=4) as esb,
        ):
            blob = const.tile([102, BLOBW], bf16)
            # staged: [cmat | chunk 0] first so compute starts early
            nc.sync.dma_start(out=blob[:, 0:M + CH], in_=blob_d[:, 0:M + CH])
            nc.sync.dma_start(out=blob[:, M + CH:BLOBW], in_=blob_d[:, M + CH:BLOBW])

            ones = const.tile([P, 1], bf16)
            nc.vector.memset(ones[:], 1.0)
            red = redp.tile([P, NRED], f32)  # one psum bank of partials

            # PE p-state warm-up: small data-independent matmuls keep the
            # tensor engine busy from t~0 so the real matmuls hit full clock
            # sooner; the target reuses a psum rotation slot that the main
            # loop only needs again ~3 tiles in.
            warm = const.tile([6, P], bf16)
            nc.vector.memset(warm[:], 0.0)
            wpt = ps.tile([P, CH], f32, tag="ps")
            for _ in range(12):
                nc.tensor.matmul(wpt[:, 0:P], warm[:], warm[:], tile_position=(0, 0))

            # Reduce matmuls for a finished tile are emitted two iterations
            # late: by then the exp pass they depend on has completed, so
            # their semaphore waits are already satisfied at decode time and
            # they never clog the PE's 4-deep wait queue (which would stall
            # the sequencer and serialize the pipeline).
            pend = []

            def flush(entry):
                et, g, c, mb = entry
                base = ((g * NCH + c) * NMB + mb) * (CH // P)
                for j in range(CH // P):
                    nc.tensor.matmul(
                        red[:, base + j:base + j + 1],
                        et[:, j * P:(j + 1) * P],
                        ones[:],
                        tile_position=(0, 0),
                    )

            for c in range(NCH):
                for g in range(NG):
                    gp = 32 * g
                    for mb in range(NMB):
                        lhsT = blob[gp:gp + 6, mb * P:(mb + 1) * P]
                        fmov = blob[gp:gp + 6, M + c * CH:M + (c + 1) * CH]
                        pt = ps.tile([P, CH], f32, tag="ps")
                        nc.tensor.matmul(pt[:, 0:HALF], lhsT, fmov[:, 0:HALF],
                                         tile_position=(gp, 0))
                        nc.tensor.matmul(pt[:, HALF:CH], lhsT, fmov[:, HALF:CH],
                                         tile_position=(gp, 0))
                        et = esb.tile([P, CH], bf16, tag="e")
                        nc.scalar.activation(
                            et[:, 0:acols], pt[:, 0:acols],
                            mybir.ActivationFunctionType.Exp,
                        )
                        nc.vector.tensor_scalar(
                            out=et[:, acols:CH].bitcast(i16),
                            in0=pt[:, acols:CH],
                            scalar1=_SCH_A, scalar2=_SCH_B,
                            op0=mybir.AluOpType.mult, op1=mybir.AluOpType.add,
                        )
                        if len(pend) >= 2:
                            flush(pend.pop(0))
                        pend.append((et, g, c, mb))
            while pend:
                flush(pend.pop(0))

            redsb = const.tile([P, NRED], f32)
            nc.vector.tensor_copy(redsb[:], red[:])
            nc.sync.dma_start(out=out_d[:], in_=redsb[:])

    nc.compile()
    return nc


def kernel(sample, mu, sigma_log, theta, w):
    import ml_dtypes

    x = sample[:, 0].astype(np.float64)
    y = sample[:, 1].astype(np.float64)
    mux = mu[:, 0].astype(np.float64)
    muy = mu[:, 1].astype(np.float64)
    sl = sigma_log.astype(np.float64)
    th = theta.astype(np.float64)
    wv = w[:, 0].astype(np.float64)

    a = np.exp(-2.0 * sl[:, 0])
    b = np.exp(-2.0 * sl[:, 1])
    c, s = np.cos(th), np.sin(th)
    g11 = a * c * c + b * s * s
    g12 = (a - b) * c * s
    g22 = a * s * s + b * c * c
    wmax = wv.max()
    wlog = (wv - (wmax + np.log(np.exp(wv - wmax).sum()))) - sl.sum(axis=1)

    # score = F @ C with F = [1, x, y, x^2, xy, y^2]
    cm = np.stack([
        wlog - (g11 * mux * mux + 2.0 * g12 * mux * muy + g22 * muy * muy),
        2.0 * (g11 * mux + g12 * muy),
        2.0 * (g12 * mux + g22 * muy),
        -g11,
        -2.0 * g12,
        -g22,
    ]).astype(np.float32)
    ftf = np.stack([np.ones_like(x), x, y, x * x, x * y, y * y]).astype(np.float32)

    cm16 = cm.astype(ml_dtypes.bfloat16)
    ftf16 = ftf.astype(ml_dtypes.bfloat16)

    if "nc" not in _cache:
        _cache["nc"] = _build()
    nc = _cache["nc"]

    in_maps = []
    for i in range(NCORES):
        blob = np.zeros((102, BLOBW), dtype=ml_dtypes.bfloat16)
        base = i * NSH
        for g in range(NG):
            gp = 32 * g
            blob[gp:gp + 6, 0:M] = cm16
            blob[gp:gp + 6, M:BLOBW] = ftf16[:, base + g * GSH:base + (g + 1) * GSH]
        in_maps.append({"blob": blob})
    res = run_bass_kernel_spmd(nc, in_maps, core_ids=list(range(NCORES)))
    _cache["last_result"] = res
    total = np.float64(0.0)
    for r in res.results:
        o = np.asarray(r["out"], dtype=np.float64)
        # columns ordered ((g, c, mb), j): sum the NMB partials per sample
        o = o.reshape(P, NG * NCH, NMB, CH // P)
        ssum = o.sum(axis=2)  # [P, g*c, j]
        total += np.log(ssum).sum()
    return np.float32(-total)
